# revision 37
# baseline (speedup 1.0000x reference)
"""Mixtral sparse-MoE block with per-expert LoRA adapters on 8 Trainium2 cores.

Problem shapes: B=2, S=1024, H=2048, F=7168, E=8, R=32, top-K=2.
T = B*S = 2048 tokens.

Sharding: tensor-parallel over the FFN dim F. Core c owns rows
[c*896:(c+1)*896] of W1/W3 (and the matching B1/B3 LoRA rows) and the same
columns of W2/A2. Everything after the silu is linear in
x2s = silu(x1)*x3*rw, so each core emits an exact partial [H, T] output over
its F-shard and the host sums the 8 partials.

Work split (device vs host):
- Host: gating (softmax + top-2; 34M MACs), the tiny per-expert LoRA
  down-projections a1/a3 = x @ A{1,3}T masked per slot (m1/m3 uploads), and
  the final LoRA up-projection lora2 = B2 @ sum_cores(m2) (one small GEMM) -
  all the terminal-linear or pre-device pieces.
- Device (per core): base1/base3 = x @ W{1,3}[shard].T, the per-slot LoRA
  up-projections lora1/3 = B{1,3}[shard] @ m{1,3} accumulated onto the base
  in PSUM, silu/mul/scale chain, a2 = A2[shard] @ x2s with per-slot masking
  (m2, returned to host), and the down-projection on the slot-summed
  activations: down = W2[shard] @ (x2s_0 + x2s_1) - the two top-k slots share
  one down-proj because down is linear in x2s.

Layout is feature-major ([feature, token]) end to end so every matmul has its
contraction on the partition axis with zero on-device transposes. All matmul
operands are bf16 (fp32 matmul is 4x slower on TRN2); PSUM accumulates fp32.

Perf notes (measured on trn2 via NTFF hardware traces):
- A dense back-to-back MM stream runs at ~216ns per K=128/N=512 bf16 matmul
  at 2.4GHz (LDWEIGHTS fully overlaps via the PE's 64-deep reorder window),
  so the kernel is PE-bound at 1680 MMs ~= 363us/core + head/tail. The chip
  sometimes sits in the P0 power state (PE 2.0GHz, everything ~20% slower).
- The block loop is software-pipelined: block b+1's phase-A groups are
  EMITTED between block b's phase-B iterations. Phase B is DVE-bound
  (~2.2us of add/silu/mul chain per f-iteration vs ~0.9us of PE work), and
  the PE executes in emission order, so each interposed phase-A group gives
  the DVE a ~7us matmul window to drain its backlog; B's PSUM leaders then
  never stall on bank release. The last block (no next A) instead weaves in
  the held-back phase-C groups of block NT-2.
- Every dma_start costs ~0.6us of ISSUE time on its HWDGE ring regardless
  of size, so streamed tensors are host-packed so one DMA feeds two chunk
  views (x/w1/w3 h-pairs, m1/m3 er-pairs), and w3 loads ride the second
  (scalar-engine) HWDGE ring. Only sync/scalar HWDGE rings are used - SWDGE
  (gpsimd) DMA breaks For_i repeat timing and is avoided.
- Outputs are unmasked a2 partials; the (elementwise) expert mask commutes
  with the cross-core sum and is applied on host before the lora2 GEMM.
"""

import sys
from contextlib import ExitStack

import numpy as np

try:
    import concourse.bass as bass  # noqa: F401
except ImportError:
    sys.path.insert(0, "/opt/trn_rl_repo")

import ml_dtypes

import concourse.bass as bass
import concourse.mybir as mybir
import concourse.tile as tile
from concourse import bacc
from concourse.bass_utils import run_bass_kernel_spmd

BF16 = mybir.dt.bfloat16
F32 = mybir.dt.float32
NPBF16 = ml_dtypes.bfloat16

B, S, H, F, E, R, K = 2, 1024, 2048, 7168, 8, 32, 2
T = B * S                      # 2048 tokens
ER = E * R                     # 256
NCORES = 8
FS = F // NCORES               # 896 per-core F shard
NH = H // 128                  # 16 h-chunks
NF = FS // 128                 # 7 f-chunks (per core)
NER = ER // 128                # 2 er-chunks
TBLK = 512
NT = T // TBLK                 # 4 token blocks


def build_nc(repeat=None):
    """Build the per-core Bass module.

    repeat=None emits the plain single-shot kernel (what the grader runs).
    repeat=n wraps the ENTIRE body — resident weight loads included — in an
    on-device For_i hardware loop for slope-based timing in test.py.
    """
    nc = bacc.Bacc(None)

    # x is host-packed in h-chunk pairs [NH//2, 128, 2, T]; m1/m3 pack
    # their two er-chunks on one axis: [K, 128, NER, T]. One DMA then
    # feeds two SBUF chunk-views, halving HWDGE issue count.
    xT = nc.declare_dram_parameter("xT", [NH // 2, 128, 2, T], BF16, isOutput=False)
    # w1/w3 are host-packed in h-chunk PAIRS: [NH//2, 128, 2*FS], pair p
    # holding h=2p in cols [0:FS) and h=2p+1 in [FS:2FS). One DMA per pair
    # halves the HWDGE issue count that gates block-0's first phase-A pass.
    w1t = nc.declare_dram_parameter("w1t", [NH // 2, 128, 2 * FS], BF16, isOutput=False)
    w3t = nc.declare_dram_parameter("w3t", [NH // 2, 128, 2 * FS], BF16, isOutput=False)
    w2t = nc.declare_dram_parameter("w2t", [NF, 128, H], BF16, isOutput=False)
    m1t = nc.declare_dram_parameter("m1t", [K, 128, NER, T], BF16, isOutput=False)
    m3t = nc.declare_dram_parameter("m3t", [K, 128, NER, T], BF16, isOutput=False)
    b1t = nc.declare_dram_parameter("b1t", [NER, 128, FS], BF16, isOutput=False)
    b3t = nc.declare_dram_parameter("b3t", [NER, 128, FS], BF16, isOutput=False)
    a2t = nc.declare_dram_parameter("a2t", [NF, 128, ER], BF16, isOutput=False)
    rwr = nc.declare_dram_parameter("rwr", [K, 1, T], BF16, isOutput=False)
    outT = nc.declare_dram_parameter("outT", [NH, 128, T], F32, isOutput=True)
    m2o = nc.declare_dram_parameter("m2o", [K, NER, 128, T], BF16, isOutput=True)

    with tile.TileContext(nc) as tc, ExitStack() as ctx:
        resw = ctx.enter_context(tc.tile_pool(name="resw", bufs=1))
        xsp = ctx.enter_context(tc.tile_pool(name="xsp", bufs=2))
        actp = ctx.enter_context(tc.tile_pool(name="actp", bufs=1))
        mp_ = ctx.enter_context(tc.tile_pool(name="mp", bufs=2))
        trans = ctx.enter_context(tc.tile_pool(name="trans", bufs=3))
        outp = ctx.enter_context(tc.tile_pool(name="outp", bufs=4))
        # PSUM partition: phase A gets 4 banks (2 tags x 2 bufs), phase B's
        # short LoRA groups 2 banks, a2 + down-proj share 2 banks. Keeping
        # the pools disjoint stops interleaved phases stealing each other's
        # bank rotations.
        psp = ctx.enter_context(tc.tile_pool(name="psp", bufs=2, space="PSUM"))
        pspB = ctx.enter_context(tc.tile_pool(name="pspB", bufs=1, space="PSUM"))
        pspD = ctx.enter_context(tc.tile_pool(name="pspD", bufs=2, space="PSUM"))

        loop_cm = tc.For_i(0, repeat, 1) if repeat is not None else None
        if loop_cm is not None:
            loop_cm.__enter__()

        # ---- per-block input streamers ----
        def load_block_inputs(tb, xs=None):
            tsl = slice(tb * TBLK, (tb + 1) * TBLK)
            if xs is None:
                xs = []
                for hp in range(NH // 2):
                    xt_ = xsp.tile([128, 2 * TBLK], BF16, name=f"x{hp}",
                                   tag=f"x{hp}")
                    nc.sync.dma_start(out=xt_, in_=xT[hp][:, :, tsl])
                    xs.append(xt_[:, 0:TBLK])
                    xs.append(xt_[:, TBLK:2 * TBLK])
            m1 = [[None] * NER for _ in range(K)]
            m3 = [[None] * NER for _ in range(K)]
            for k in range(K):
                m1_ = mp_.tile([128, 2 * TBLK], BF16, name=f"m1_{k}",
                               tag=f"m1_{k}")
                nc.sync.dma_start(out=m1_, in_=m1t[k][:, :, tsl])
                m3_ = mp_.tile([128, 2 * TBLK], BF16, name=f"m3_{k}",
                               tag=f"m3_{k}")
                nc.sync.dma_start(out=m3_, in_=m3t[k][:, :, tsl])
                for er in range(NER):
                    m1[k][er] = m1_[:, er * TBLK:(er + 1) * TBLK]
                    m3[k][er] = m3_[:, er * TBLK:(er + 1) * TBLK]
            rws = []
            for k in range(K):
                r_ = mp_.tile([128, TBLK], BF16, name=f"rw{k}", tag=f"rw{k}")
                nc.sync.dma_start(out=r_, in_=rwr[k][:, tsl].to_broadcast([128, TBLK]))
                rws.append(r_)
            return xs, rws, m1, m3

        # ---- resident weights, emitted in first-use order so the HWDGE
        # queue feeds phase A of block 0 as early as possible ----
        def resident(src, n, shape, nm, eng=None):
            ts = []
            for i in range(n):
                t_ = resw.tile(shape, BF16, name=f"{nm}{i}", tag=f"{nm}{i}")
                (eng or nc.sync).dma_start(out=t_, in_=src[i])
                ts.append(t_)
            return ts

        xs0 = []
        w1p, w3p = [], []
        for hp in range(NH // 2):
            # alternate x pairs across the two HWDGE rings so block-0's
            # first phase-A pass is supply-balanced (~0.61us issue per DMA)
            xt_ = xsp.tile([128, 2 * TBLK], BF16, name=f"x{hp}", tag=f"x{hp}")
            xeng = nc.sync if hp % 2 == 0 else nc.scalar
            xeng.dma_start(out=xt_, in_=xT[hp][:, :, 0:TBLK])
            xs0.append(xt_[:, 0:TBLK])
            xs0.append(xt_[:, TBLK:2 * TBLK])
            t1 = resw.tile([128, 2 * FS], BF16, name=f"w1s{hp}", tag=f"w1s{hp}")
            nc.sync.dma_start(out=t1, in_=w1t[hp])
            w1p.append(t1)
            t3 = resw.tile([128, 2 * FS], BF16, name=f"w3s{hp}", tag=f"w3s{hp}")
            nc.scalar.dma_start(out=t3, in_=w3t[hp])
            w3p.append(t3)
        # per-h weight views into the packed pair tiles
        w1s = [w1p[h // 2][:, (h % 2) * FS:(h % 2) * FS + FS] for h in range(NH)]
        w3s = [w3p[h // 2][:, (h % 2) * FS:(h % 2) * FS + FS] for h in range(NH)]
        pre0 = load_block_inputs(0, xs0)
        b1s = resident(b1t, NER, [128, FS], "b1s")
        b3s = resident(b3t, NER, [128, FS], "b3s")
        w2s = resident(w2t, NF, [128, H], "w2s")
        a2s = resident(a2t, NF, [128, ER], "a2s")

        # ---- phase emitters (software-pipelined across blocks below) ----
        def emit_A_group(xs, f, base1, base3):
            """One f-chunk of base1/base3 = W1/W3 @ x (PE-dense, no deps)."""
            fsl = slice(f * 128, (f + 1) * 128)
            ps1 = psp.tile([128, TBLK], F32, name="ps1", tag="pA")
            ps3 = psp.tile([128, TBLK], F32, name="ps3", tag="pB")
            for h in range(NH):
                nc.tensor.matmul(ps1, w1s[h][:, fsl], xs[h], start=(h == 0), stop=(h == NH - 1))
                nc.tensor.matmul(ps3, w3s[h][:, fsl], xs[h], start=(h == 0), stop=(h == NH - 1))
            b1_ = actp.tile([128, TBLK], BF16, name=f"b1_{f}", tag=f"b1_{f}")
            nc.scalar.copy(b1_, ps1)
            base1[f] = b1_
            b3_ = actp.tile([128, TBLK], BF16, name=f"b3_{f}", tag=f"b3_{f}")
            nc.scalar.copy(b3_, ps3)
            base3[f] = b3_

        def emit_A(xs):
            base1, base3 = [None] * NF, [None] * NF
            for f in range(NF):
                emit_A_group(xs, f, base1, base3)
            return base1, base3

        def emit_xsum(f, x2s, xsum):
            xs_ = actp.tile([128, TBLK], BF16, name=f"xsum{f}",
                            tag=f"xsum{f}")
            nc.gpsimd.tensor_add(xs_, x2s[0][f], x2s[1][f])
            xsum[f] = xs_

        def emit_B_f(k, f, base1, base3, rws, m1, m3, x2s, xsum,
                     do_xsum=True):
            """LoRA up-proj + silu/mul chain for one (slot, f-chunk)."""
            fsl = slice(f * 128, (f + 1) * 128)
            psA = pspB.tile([128, TBLK], F32, name="psA", tag="qA")
            nc.tensor.matmul(psA, b1s[0][:, fsl], m1[k][0], start=True, stop=False)
            nc.tensor.matmul(psA, b1s[1][:, fsl], m1[k][1], start=False, stop=True)
            psB = pspB.tile([128, TBLK], F32, name="psB", tag="qB")
            nc.tensor.matmul(psB, b3s[0][:, fsl], m3[k][0], start=True, stop=False)
            nc.tensor.matmul(psB, b3s[1][:, fsl], m3[k][1], start=False, stop=True)
            t1_ = trans.tile([128, TBLK], BF16, name="t1", tag="t1")
            nc.vector.tensor_add(t1_, psA, base1[f])
            sl_ = trans.tile([128, TBLK], BF16, name="sl", tag="sl")
            nc.scalar.activation(sl_, t1_, mybir.ActivationFunctionType.Silu)
            t3_ = trans.tile([128, TBLK], BF16, name="t3", tag="t3")
            nc.vector.tensor_add(t3_, psB, base3[f])
            x3s_ = trans.tile([128, TBLK], BF16, name="x3s", tag="x3s")
            nc.vector.tensor_mul(x3s_, t3_, rws[k])
            x2_ = actp.tile([128, TBLK], BF16, name=f"x2_{k}{f}",
                            tag=f"x2_{k}{f}")
            nc.vector.tensor_mul(x2_, sl_, x3s_)
            x2s[k][f] = x2_
            if k == K - 1 and do_xsum:
                emit_xsum(f, x2s, xsum)

        def emit_a2(tb, k, x2s):
            tsl = slice(tb * TBLK, (tb + 1) * TBLK)
            for er in range(NER):
                ers = slice(er * 128, (er + 1) * 128)
                psa2 = pspD.tile([128, TBLK], F32, name="psa2", tag="pD")
                for f in range(NF):
                    nc.tensor.matmul(psa2, a2s[f][:, ers], x2s[k][f],
                                     start=(f == 0), stop=(f == NF - 1))
                m2_ = actp.tile([128, TBLK], BF16, name=f"m2_{k}{er}",
                                tag=f"m2_{k}{er}")
                nc.scalar.copy(m2_, psa2)
                nc.sync.dma_start(out=m2o[k][er][:, tsl], in_=m2_)

        def emit_C_group(tb, xsum, h):
            tsl = slice(tb * TBLK, (tb + 1) * TBLK)
            hsl = slice(h * 128, (h + 1) * 128)
            psD = pspD.tile([128, TBLK], F32, name="psD", tag="pD")
            for f in range(NF):
                nc.tensor.matmul(psD, w2s[f][:, hsl], xsum[f],
                                 start=(f == 0), stop=(f == NF - 1))
            o_ = outp.tile([128, TBLK], F32, name="osb", tag="osb")
            nc.scalar.copy(o_, psD)
            nc.sync.dma_start(out=outT[h][:, tsl], in_=o_)

        def emit_C(tb, xsum, last=False):
            for h in range(NH):
                emit_C_group(tb, xsum, h)

        # ---- software pipeline: next block's phase-A groups are woven
        # BETWEEN this block's phase-B iterations (PE executes in emission
        # order, so independent work must be emitted before gated work).
        # Phase B is Vector-throughput-bound (~2.2us of DVE chain per
        # f-iteration vs ~0.9us of PE); each interposed A group gives the
        # DVE ~7us of matmul cover to drain its chain backlog, so the
        # B-group PSUM leaders never wait on bank release. ----
        xs, rws, m1, m3 = pre0
        base1, base3 = emit_A(xs)
        heldC = None   # xsum of block NT-2, its C woven into the last block
        for tb in range(NT):
            x2s = [[None] * NF for _ in range(K)]
            xsum = [None] * NF
            B = lambda k, f: emit_B_f(k, f, base1, base3, rws, m1, m3,
                                      x2s, xsum)
            if tb + 1 < NT:
                xsn, rwsn, m1n, m3n = load_block_inputs(tb + 1)
                b1n, b3n = [None] * NF, [None] * NF
                A = lambda f: emit_A_group(xsn, f, b1n, b3n)
                B(0, 0); B(0, 1)
                B(0, 2); B(0, 3); A(0)
                B(0, 4); B(0, 5); A(1)
                B(0, 6); B(1, 0); A(2)
                B(1, 1); B(1, 2); A(3)
                B(1, 3); B(1, 4); emit_a2(tb, 0, x2s)
                B(1, 5); B(1, 6); A(4)
                emit_a2(tb, 1, x2s)
                A(5); A(6)
                xs, rws, m1, m3 = xsn, rwsn, m1n, m3n
                base1, base3 = b1n, b3n
                if tb == NT - 2:
                    heldC = xsum      # defer C(NT-2) into the last block
                else:
                    emit_C(tb, xsum)
            else:
                # last block has no next-A cover; weave the held-back
                # C(NT-2) groups among the k=0 iterations instead. All held
                # groups must be emitted before B(1,0) writes xsum (the
                # single-buffered xsum tags roll over to this block there).
                hq = list(range(NH))
                C2 = lambda n: [emit_C_group(tb - 1, heldC, hq.pop(0))
                                for _ in range(n)]
                Bx = lambda k, f: emit_B_f(k, f, base1, base3, rws, m1, m3,
                                           x2s, xsum, do_xsum=False)
                Bx(0, 0); Bx(1, 0); C2(1)
                Bx(0, 1); Bx(1, 1); C2(1)
                Bx(0, 2); Bx(1, 2); C2(2)
                Bx(0, 3); Bx(1, 3); C2(2)
                Bx(0, 4); Bx(1, 4); C2(2)
                Bx(0, 5); Bx(1, 5); C2(4)
                Bx(0, 6); Bx(1, 6); C2(4)
                for f in range(NF):
                    emit_xsum(f, x2s, xsum)
                emit_a2(tb, 0, x2s)
                emit_a2(tb, 1, x2s)
                emit_C(tb, xsum, last=True)

        if loop_cm is not None:
            loop_cm.__exit__(None, None, None)

    nc.finalize()
    return nc


def prepare_inputs(hidden_states, Wg, W1, W2, W3, A1, B1, A2, B2, A3, B3):
    """Host preprocessing: routing + per-core weight slicing/casting."""
    hidden_states, Wg, W1, W2, W3, A1, B1, A2, B2, A3, B3 = (
        np.asarray(a, dtype=np.float32)
        for a in (hidden_states, Wg, W1, W2, W3, A1, B1, A2, B2, A3, B3))
    x = np.ascontiguousarray(hidden_states.reshape(T, H))

    logits = x @ Wg.T.astype(np.float32)
    m = logits.max(-1, keepdims=True)
    p = np.exp(logits - m, dtype=np.float32)
    p /= p.sum(-1, keepdims=True)
    sel = np.argsort(-p, axis=-1, kind="stable")[:, :K]      # [T, K]
    rw = np.take_along_axis(p, sel, axis=1)
    rw = (rw / rw.sum(-1, keepdims=True)).astype(np.float32)  # [T, K]

    xT_np = np.ascontiguousarray(
        x.T.reshape(NH // 2, 2, 128, T).transpose(0, 2, 1, 3)
    ).astype(NPBF16)                                  # [NH//2, 128, 2, T]

    # per-slot one-hot masks over the (e, r) axis, transposed to [ER, T];
    # applied HOST-side to the returned a2 (masking is elementwise, so it
    # commutes with the cross-core partial sum)
    masks = np.zeros((K, ER, T), dtype=np.float32)
    for k in range(K):
        onehot = np.zeros((T, E), np.float32)
        onehot[np.arange(T), sel[:, k]] = 1.0
        masks[k] = np.repeat(onehot, R, axis=1).T
    rwr_np = np.ascontiguousarray(rw.T).reshape(K, 1, T).astype(NPBF16)

    # flattened LoRA tensors (full copies; small)
    A1f = A1.reshape(ER, H)                      # [er, H]
    A3f = A3.reshape(ER, H)
    B2f = B2.transpose(0, 2, 1).reshape(ER, H)   # [er, H]

    # per-slot masked LoRA down-projections, computed host-side in fp32
    a1_all = x @ A1f.T.astype(np.float32)        # [T, ER]
    a3_all = x @ A3f.T.astype(np.float32)
    m1t_np = np.zeros((K, ER, T), dtype=NPBF16)
    m3t_np = np.zeros((K, ER, T), dtype=NPBF16)
    for k in range(K):
        mx = np.repeat(
            np.eye(E, dtype=np.float32)[sel[:, k]], R, axis=1)   # [T, ER]
        m1t_np[k] = (a1_all * mx).T.astype(NPBF16)
        m3t_np[k] = (a3_all * mx).T.astype(NPBF16)
    m1t_np = np.ascontiguousarray(
        m1t_np.reshape(K, NER, 128, T).transpose(0, 2, 1, 3))
    m3t_np = np.ascontiguousarray(
        m3t_np.reshape(K, NER, 128, T).transpose(0, 2, 1, 3))

    def pack_pairs(wT):
        # [H, FS] -> [NH//2, 128, 2*FS] with h=2p at cols [0:FS), h=2p+1
        # at [FS:2FS) (matches the kernel's paired w1t/w3t layout)
        return np.ascontiguousarray(
            wT.reshape(NH // 2, 2, 128, FS).transpose(0, 2, 1, 3)
        ).reshape(NH // 2, 128, 2 * FS)

    in_maps = []
    for c in range(NCORES):
        fs = slice(c * FS, (c + 1) * FS)
        w1T = np.ascontiguousarray(W1[fs].T).astype(NPBF16)   # [H, FS]
        w3T = np.ascontiguousarray(W3[fs].T).astype(NPBF16)
        w1t_np = pack_pairs(w1T)
        w3t_np = pack_pairs(w3T)
        w2T = np.ascontiguousarray(W2[:, fs].T).astype(NPBF16)  # [FS, H]
        w2t_np = w2T.reshape(NF, 128, H)
        b1f = B1[:, fs, :].transpose(0, 2, 1).reshape(ER, FS)   # [er, f]
        b3f = B3[:, fs, :].transpose(0, 2, 1).reshape(ER, FS)
        b1t_np = np.ascontiguousarray(b1f).astype(NPBF16).reshape(NER, 128, FS)
        b3t_np = np.ascontiguousarray(b3f).astype(NPBF16).reshape(NER, 128, FS)
        a2f = A2[:, :, fs].reshape(ER, FS)                      # [er, f]
        a2t_np = np.ascontiguousarray(a2f.T).astype(NPBF16).reshape(NF, 128, ER)

        in_maps.append({
            "xT": xT_np, "w1t": w1t_np, "w3t": w3t_np, "w2t": w2t_np,
            "m1t": m1t_np, "m3t": m3t_np, "b1t": b1t_np, "b3t": b3t_np,
            "a2t": a2t_np,
            "rwr": rwr_np,
        })
    return in_maps, (B2f.astype(np.float32), masks)


_CACHED_NC = None


def kernel(hidden_states, Wg, W1, W2, W3, A1, B1, A2, B2, A3, B3,
           _trace=False, _tmpdir=None):
    global _CACHED_NC
    in_maps, (B2f, masks) = prepare_inputs(hidden_states, Wg, W1, W2, W3,
                                           A1, B1, A2, B2, A3, B3)
    if _CACHED_NC is None:
        _CACHED_NC = build_nc()
    nc = _CACHED_NC
    res = run_bass_kernel_spmd(nc, in_maps, list(range(NCORES)),
                               trace=_trace, tmpdir=_tmpdir)
    acc = np.zeros((NH, 128, T), np.float32)
    m2sum = np.zeros((K, ER, T), np.float32)
    for c in range(NCORES):
        acc += res.results[c]["outT"]
        m2sum += res.results[c]["m2o"].reshape(K, ER, T).astype(np.float32)
    out = acc.reshape(H, T)
    # host-side lora2: mask the (unmasked, core-summed) a2, then the final
    # LoRA up-projection is linear -> one small GEMM per slot
    for k in range(K):
        out += B2f.T @ (m2sum[k] * masks[k])
    out = out.T.reshape(B, S, H)
    kernel.last_results = res
    return out


if __name__ == "__main__":
    nc = build_nc()
    print("built ok")



# revision 39
# speedup vs baseline: 1.0149x; 1.0149x over previous
"""Mixtral sparse-MoE block with per-expert LoRA adapters on 8 Trainium2 cores.

Problem shapes: B=2, S=1024, H=2048, F=7168, E=8, R=32, top-K=2.
T = B*S = 2048 tokens.

Sharding: tensor-parallel over the FFN dim F. Core c owns rows
[c*896:(c+1)*896] of W1/W3 (and the matching B1/B3 LoRA rows) and the same
columns of W2/A2. Everything after the silu is linear in
x2s = silu(x1)*x3*rw, so each core emits an exact partial [H, T] output over
its F-shard and the host sums the 8 partials.

Work split (device vs host):
- Host: gating (softmax + top-2; 34M MACs), the tiny per-expert LoRA
  down-projections a1/a3 = x @ A{1,3}T masked per slot (m1/m3 uploads), and
  the final LoRA up-projection lora2 = B2 @ sum_cores(m2) (one small GEMM) -
  all the terminal-linear or pre-device pieces.
- Device (per core): base1/base3 = x @ W{1,3}[shard].T, the per-slot LoRA
  up-projections lora1/3 = B{1,3}[shard] @ m{1,3} accumulated onto the base
  in PSUM, silu/mul/scale chain, a2 = A2[shard] @ x2s with per-slot masking
  (m2, returned to host), and the down-projection on the slot-summed
  activations: down = W2[shard] @ (x2s_0 + x2s_1) - the two top-k slots share
  one down-proj because down is linear in x2s.

Layout is feature-major ([feature, token]) end to end so every matmul has its
contraction on the partition axis with zero on-device transposes. All matmul
operands are bf16 (fp32 matmul is 4x slower on TRN2); PSUM accumulates fp32.

Perf notes (measured on trn2 via NTFF hardware traces):
- A dense back-to-back MM stream runs at ~216ns per K=128/N=512 bf16 matmul
  at 2.4GHz (LDWEIGHTS fully overlaps via the PE's 64-deep reorder window),
  so the kernel is PE-bound at 1680 MMs ~= 363us/core + head/tail. The chip
  sometimes sits in the P0 power state (PE 2.0GHz, everything ~20% slower).
- The block loop is software-pipelined: block b+1's phase-A groups are
  EMITTED between block b's phase-B iterations. Phase B is DVE-bound
  (~2.2us of add/silu/mul chain per f-iteration vs ~0.9us of PE work), and
  the PE executes in emission order, so each interposed phase-A group gives
  the DVE a ~7us matmul window to drain its backlog; B's PSUM leaders then
  never stall on bank release. The last block (no next A) instead weaves in
  the held-back phase-C groups of block NT-2.
- Every dma_start costs ~0.6us of ISSUE time on its HWDGE ring regardless
  of size, so streamed tensors are host-packed so one DMA feeds two chunk
  views (x/w1/w3 h-pairs, m1/m3 er-pairs), and w3 loads ride the second
  (scalar-engine) HWDGE ring. Only sync/scalar HWDGE rings are used - SWDGE
  (gpsimd) DMA breaks For_i repeat timing and is avoided.
- Outputs are unmasked a2 partials; the (elementwise) expert mask commutes
  with the cross-core sum and is applied on host before the lora2 GEMM.
"""

import sys
from contextlib import ExitStack

import numpy as np

try:
    import concourse.bass as bass  # noqa: F401
except ImportError:
    sys.path.insert(0, "/opt/trn_rl_repo")

import ml_dtypes

import concourse.bass as bass
import concourse.mybir as mybir
import concourse.tile as tile
from concourse import bacc
from concourse.bass_utils import run_bass_kernel_spmd

BF16 = mybir.dt.bfloat16
F32 = mybir.dt.float32
NPBF16 = ml_dtypes.bfloat16

B, S, H, F, E, R, K = 2, 1024, 2048, 7168, 8, 32, 2
T = B * S                      # 2048 tokens
ER = E * R                     # 256
NCORES = 8
FS = F // NCORES               # 896 per-core F shard
NH = H // 128                  # 16 h-chunks
NF = FS // 128                 # 7 f-chunks (per core)
NER = ER // 128                # 2 er-chunks
TBLK = 512
NT = T // TBLK                 # 4 token blocks


def build_nc(repeat=None, sparse=False):
    """Build the per-core Bass module.

    repeat=None emits the plain single-shot kernel (what the grader runs).
    repeat=n wraps the ENTIRE body in an on-device For_i hardware loop.
    sparse=True assumes the host permuted tokens by slot-0 expert GROUP
    (experts 0-3 first): block 0 then only needs er-chunk 0 for slot 0 and
    block NT-1 only er-chunk 1 (the other chunk's m1/m3 columns are zero
    and its a2 rows are masked out) — 42 fewer matmuls. prepare_inputs
    verifies the purity guard and falls back to the dense variant.
    """
    nc = bacc.Bacc(None)

    # x is host-packed in h-chunk pairs [NH//2, 128, 2, T]; m1/m3 pack
    # their two er-chunks on one axis: [K, 128, NER, T]. One DMA then
    # feeds two SBUF chunk-views, halving HWDGE issue count.
    xT = nc.declare_dram_parameter("xT", [NH // 2, 128, 2, T], BF16, isOutput=False)
    # w1/w3 are host-packed in h-chunk PAIRS: [NH//2, 128, 2*FS], pair p
    # holding h=2p in cols [0:FS) and h=2p+1 in [FS:2FS). One DMA per pair
    # halves the HWDGE issue count that gates block-0's first phase-A pass.
    w1t = nc.declare_dram_parameter("w1t", [NH // 2, 128, 2 * FS], BF16, isOutput=False)
    w3t = nc.declare_dram_parameter("w3t", [NH // 2, 128, 2 * FS], BF16, isOutput=False)
    w2t = nc.declare_dram_parameter("w2t", [NF, 128, H], BF16, isOutput=False)
    m1t = nc.declare_dram_parameter("m1t", [K, 128, NER, T], BF16, isOutput=False)
    m3t = nc.declare_dram_parameter("m3t", [K, 128, NER, T], BF16, isOutput=False)
    b1t = nc.declare_dram_parameter("b1t", [NER, 128, FS], BF16, isOutput=False)
    b3t = nc.declare_dram_parameter("b3t", [NER, 128, FS], BF16, isOutput=False)
    a2t = nc.declare_dram_parameter("a2t", [NF, 128, ER], BF16, isOutput=False)
    rwr = nc.declare_dram_parameter("rwr", [K, 1, T], BF16, isOutput=False)
    outT = nc.declare_dram_parameter("outT", [NH, 128, T], F32, isOutput=True)
    m2o = nc.declare_dram_parameter("m2o", [K, NER, 128, T], BF16, isOutput=True)

    with tile.TileContext(nc) as tc, ExitStack() as ctx:
        resw = ctx.enter_context(tc.tile_pool(name="resw", bufs=1))
        xsp = ctx.enter_context(tc.tile_pool(name="xsp", bufs=2))
        actp = ctx.enter_context(tc.tile_pool(name="actp", bufs=1))
        mp_ = ctx.enter_context(tc.tile_pool(name="mp", bufs=2))
        trans = ctx.enter_context(tc.tile_pool(name="trans", bufs=3))
        outp = ctx.enter_context(tc.tile_pool(name="outp", bufs=4))
        # PSUM partition: phase A gets 4 banks (2 tags x 2 bufs), phase B's
        # short LoRA groups 2 banks, a2 + down-proj share 2 banks. Keeping
        # the pools disjoint stops interleaved phases stealing each other's
        # bank rotations.
        psp = ctx.enter_context(tc.tile_pool(name="psp", bufs=2, space="PSUM"))
        pspB = ctx.enter_context(tc.tile_pool(name="pspB", bufs=1, space="PSUM"))
        pspD = ctx.enter_context(tc.tile_pool(name="pspD", bufs=2, space="PSUM"))

        loop_cm = tc.For_i(0, repeat, 1) if repeat is not None else None
        if loop_cm is not None:
            loop_cm.__enter__()

        # ---- per-block input streamers ----
        def load_block_inputs(tb, xs=None):
            tsl = slice(tb * TBLK, (tb + 1) * TBLK)
            if xs is None:
                xs = []
                for hp in range(NH // 2):
                    xt_ = xsp.tile([128, 2 * TBLK], BF16, name=f"x{hp}",
                                   tag=f"x{hp}")
                    nc.sync.dma_start(out=xt_, in_=xT[hp][:, :, tsl])
                    xs.append(xt_[:, 0:TBLK])
                    xs.append(xt_[:, TBLK:2 * TBLK])
            m1 = [[None] * NER for _ in range(K)]
            m3 = [[None] * NER for _ in range(K)]
            for k in range(K):
                m1_ = mp_.tile([128, 2 * TBLK], BF16, name=f"m1_{k}",
                               tag=f"m1_{k}")
                nc.sync.dma_start(out=m1_, in_=m1t[k][:, :, tsl])
                m3_ = mp_.tile([128, 2 * TBLK], BF16, name=f"m3_{k}",
                               tag=f"m3_{k}")
                nc.sync.dma_start(out=m3_, in_=m3t[k][:, :, tsl])
                for er in range(NER):
                    m1[k][er] = m1_[:, er * TBLK:(er + 1) * TBLK]
                    m3[k][er] = m3_[:, er * TBLK:(er + 1) * TBLK]
            rws = []
            for k in range(K):
                r_ = mp_.tile([128, TBLK], BF16, name=f"rw{k}", tag=f"rw{k}")
                nc.sync.dma_start(out=r_, in_=rwr[k][:, tsl].to_broadcast([128, TBLK]))
                rws.append(r_)
            return xs, rws, m1, m3

        # ---- resident weights, emitted in first-use order so the HWDGE
        # queue feeds phase A of block 0 as early as possible ----
        def resident(src, n, shape, nm, eng=None):
            ts = []
            for i in range(n):
                t_ = resw.tile(shape, BF16, name=f"{nm}{i}", tag=f"{nm}{i}")
                (eng or nc.sync).dma_start(out=t_, in_=src[i])
                ts.append(t_)
            return ts

        xs0 = []
        w1p, w3p = [], []
        for hp in range(NH // 2):
            # alternate x pairs across the two HWDGE rings so block-0's
            # first phase-A pass is supply-balanced (~0.61us issue per DMA)
            xt_ = xsp.tile([128, 2 * TBLK], BF16, name=f"x{hp}", tag=f"x{hp}")
            xeng = nc.sync if hp % 2 == 0 else nc.scalar
            xeng.dma_start(out=xt_, in_=xT[hp][:, :, 0:TBLK])
            xs0.append(xt_[:, 0:TBLK])
            xs0.append(xt_[:, TBLK:2 * TBLK])
            t1 = resw.tile([128, 2 * FS], BF16, name=f"w1s{hp}", tag=f"w1s{hp}")
            nc.sync.dma_start(out=t1, in_=w1t[hp])
            w1p.append(t1)
            t3 = resw.tile([128, 2 * FS], BF16, name=f"w3s{hp}", tag=f"w3s{hp}")
            nc.scalar.dma_start(out=t3, in_=w3t[hp])
            w3p.append(t3)
        # per-h weight views into the packed pair tiles
        w1s = [w1p[h // 2][:, (h % 2) * FS:(h % 2) * FS + FS] for h in range(NH)]
        w3s = [w3p[h // 2][:, (h % 2) * FS:(h % 2) * FS + FS] for h in range(NH)]
        pre0 = load_block_inputs(0, xs0)
        b1s = resident(b1t, NER, [128, FS], "b1s")
        b3s = resident(b3t, NER, [128, FS], "b3s")
        w2s = resident(w2t, NF, [128, H], "w2s")
        a2s = resident(a2t, NF, [128, ER], "a2s")

        # ---- phase emitters (software-pipelined across blocks below) ----
        def emit_A_group(xs, f, base1, base3):
            """One f-chunk of base1/base3 = W1/W3 @ x (PE-dense, no deps)."""
            fsl = slice(f * 128, (f + 1) * 128)
            ps1 = psp.tile([128, TBLK], F32, name="ps1", tag="pA")
            ps3 = psp.tile([128, TBLK], F32, name="ps3", tag="pB")
            for h in range(NH):
                nc.tensor.matmul(ps1, w1s[h][:, fsl], xs[h], start=(h == 0), stop=(h == NH - 1))
                nc.tensor.matmul(ps3, w3s[h][:, fsl], xs[h], start=(h == 0), stop=(h == NH - 1))
            b1_ = actp.tile([128, TBLK], BF16, name=f"b1_{f}", tag=f"b1_{f}")
            nc.scalar.copy(b1_, ps1)
            base1[f] = b1_
            b3_ = actp.tile([128, TBLK], BF16, name=f"b3_{f}", tag=f"b3_{f}")
            nc.scalar.copy(b3_, ps3)
            base3[f] = b3_

        def emit_A(xs):
            base1, base3 = [None] * NF, [None] * NF
            for f in range(NF):
                emit_A_group(xs, f, base1, base3)
            return base1, base3

        def emit_xsum(f, x2s, xsum):
            xs_ = actp.tile([128, TBLK], BF16, name=f"xsum{f}",
                            tag=f"xsum{f}")
            nc.gpsimd.tensor_add(xs_, x2s[0][f], x2s[1][f])
            xsum[f] = xs_

        def active_ers(tb, k):
            if not sparse or k != 0:
                return (0, 1)
            if tb == 0:
                return (0,)
            if tb == NT - 1:
                return (1,)
            return (0, 1)

        def emit_B_f(tb, k, f, base1, base3, rws, m1, m3, x2s, xsum,
                     do_xsum=True):
            """LoRA up-proj + silu/mul chain for one (slot, f-chunk)."""
            acts = active_ers(tb, k)
            fsl = slice(f * 128, (f + 1) * 128)
            psA = pspB.tile([128, TBLK], F32, name="psA", tag="qA")
            for i, er in enumerate(acts):
                nc.tensor.matmul(psA, b1s[er][:, fsl], m1[k][er],
                                 start=(i == 0), stop=(i == len(acts) - 1))
            psB = pspB.tile([128, TBLK], F32, name="psB", tag="qB")
            for i, er in enumerate(acts):
                nc.tensor.matmul(psB, b3s[er][:, fsl], m3[k][er],
                                 start=(i == 0), stop=(i == len(acts) - 1))
            t1_ = trans.tile([128, TBLK], BF16, name="t1", tag="t1")
            nc.vector.tensor_add(t1_, psA, base1[f])
            sl_ = trans.tile([128, TBLK], BF16, name="sl", tag="sl")
            nc.scalar.activation(sl_, t1_, mybir.ActivationFunctionType.Silu)
            t3_ = trans.tile([128, TBLK], BF16, name="t3", tag="t3")
            nc.vector.tensor_add(t3_, psB, base3[f])
            x3s_ = trans.tile([128, TBLK], BF16, name="x3s", tag="x3s")
            nc.vector.tensor_mul(x3s_, t3_, rws[k])
            x2_ = actp.tile([128, TBLK], BF16, name=f"x2_{k}{f}",
                            tag=f"x2_{k}{f}")
            nc.vector.tensor_mul(x2_, sl_, x3s_)
            x2s[k][f] = x2_
            if k == K - 1 and do_xsum:
                emit_xsum(f, x2s, xsum)

        def emit_a2(tb, k, x2s):
            tsl = slice(tb * TBLK, (tb + 1) * TBLK)
            for er in active_ers(tb, k):
                ers = slice(er * 128, (er + 1) * 128)
                psa2 = pspD.tile([128, TBLK], F32, name="psa2", tag="pD")
                for f in range(NF):
                    nc.tensor.matmul(psa2, a2s[f][:, ers], x2s[k][f],
                                     start=(f == 0), stop=(f == NF - 1))
                m2_ = actp.tile([128, TBLK], BF16, name=f"m2_{k}{er}",
                                tag=f"m2_{k}{er}")
                nc.scalar.copy(m2_, psa2)
                nc.sync.dma_start(out=m2o[k][er][:, tsl], in_=m2_)

        def emit_C_group(tb, xsum, h):
            tsl = slice(tb * TBLK, (tb + 1) * TBLK)
            hsl = slice(h * 128, (h + 1) * 128)
            psD = pspD.tile([128, TBLK], F32, name="psD", tag="pD")
            for f in range(NF):
                nc.tensor.matmul(psD, w2s[f][:, hsl], xsum[f],
                                 start=(f == 0), stop=(f == NF - 1))
            o_ = outp.tile([128, TBLK], F32, name="osb", tag="osb")
            nc.scalar.copy(o_, psD)
            nc.sync.dma_start(out=outT[h][:, tsl], in_=o_)

        def emit_C(tb, xsum, last=False):
            for h in range(NH):
                emit_C_group(tb, xsum, h)

        # ---- software pipeline: next block's phase-A groups are woven
        # BETWEEN this block's phase-B iterations (PE executes in emission
        # order, so independent work must be emitted before gated work).
        # Phase B is Vector-throughput-bound (~2.2us of DVE chain per
        # f-iteration vs ~0.9us of PE); each interposed A group gives the
        # DVE ~7us of matmul cover to drain its chain backlog, so the
        # B-group PSUM leaders never wait on bank release. ----
        xs, rws, m1, m3 = pre0
        base1, base3 = emit_A(xs)
        heldC = None   # xsum of block NT-2, its C woven into the last block
        for tb in range(NT):
            x2s = [[None] * NF for _ in range(K)]
            xsum = [None] * NF
            B = lambda k, f: emit_B_f(tb, k, f, base1, base3, rws, m1, m3,
                                      x2s, xsum)
            if tb + 1 < NT:
                xsn, rwsn, m1n, m3n = load_block_inputs(tb + 1)
                b1n, b3n = [None] * NF, [None] * NF
                A = lambda f: emit_A_group(xsn, f, b1n, b3n)
                B(0, 0); B(0, 1)
                B(0, 2); B(0, 3); A(0)
                B(0, 4); B(0, 5); A(1)
                B(0, 6); B(1, 0); A(2)
                B(1, 1); B(1, 2); A(3)
                B(1, 3); B(1, 4); emit_a2(tb, 0, x2s)
                B(1, 5); B(1, 6); A(4)
                emit_a2(tb, 1, x2s)
                A(5); A(6)
                xs, rws, m1, m3 = xsn, rwsn, m1n, m3n
                base1, base3 = b1n, b3n
                if tb == NT - 2:
                    heldC = xsum      # defer C(NT-2) into the last block
                else:
                    emit_C(tb, xsum)
            else:
                # last block has no next-A cover; weave the held-back
                # C(NT-2) groups among the k=0 iterations instead. All held
                # groups must be emitted before B(1,0) writes xsum (the
                # single-buffered xsum tags roll over to this block there).
                hq = list(range(NH))
                C2 = lambda n: [emit_C_group(tb - 1, heldC, hq.pop(0))
                                for _ in range(n)]
                Bx = lambda k, f: emit_B_f(tb, k, f, base1, base3, rws, m1,
                                           m3, x2s, xsum, do_xsum=False)
                Bx(0, 0); Bx(1, 0); C2(1)
                Bx(0, 1); Bx(1, 1); C2(1)
                Bx(0, 2); Bx(1, 2); C2(2)
                Bx(0, 3); Bx(1, 3); C2(2)
                Bx(0, 4); Bx(1, 4); C2(2)
                Bx(0, 5); Bx(1, 5); C2(4)
                Bx(0, 6); Bx(1, 6); C2(4)
                for f in range(NF):
                    emit_xsum(f, x2s, xsum)
                emit_a2(tb, 0, x2s)
                emit_a2(tb, 1, x2s)
                emit_C(tb, xsum, last=True)

        if loop_cm is not None:
            loop_cm.__exit__(None, None, None)

    nc.finalize()
    return nc


def prepare_inputs(hidden_states, Wg, W1, W2, W3, A1, B1, A2, B2, A3, B3):
    """Host preprocessing: routing + per-core weight slicing/casting."""
    hidden_states, Wg, W1, W2, W3, A1, B1, A2, B2, A3, B3 = (
        np.asarray(a, dtype=np.float32)
        for a in (hidden_states, Wg, W1, W2, W3, A1, B1, A2, B2, A3, B3))
    x = np.ascontiguousarray(hidden_states.reshape(T, H))

    logits = x @ Wg.T.astype(np.float32)
    m = logits.max(-1, keepdims=True)
    p = np.exp(logits - m, dtype=np.float32)
    p /= p.sum(-1, keepdims=True)
    sel = np.argsort(-p, axis=-1, kind="stable")[:, :K]      # [T, K]
    rw = np.take_along_axis(p, sel, axis=1)
    rw = (rw / rw.sum(-1, keepdims=True)).astype(np.float32)  # [T, K]

    # Sort tokens by slot-0 expert GROUP (experts 0-3 first). When the
    # first and last TBLK sorted tokens are group-pure (holds w.h.p. for
    # balanced routing), the device can skip the zero er-chunk of the
    # slot-0 LoRA in blocks 0 and NT-1 (the sparse build_nc variant).
    grp = (sel[:, 0] >= E // 2).astype(np.int64)
    perm = np.argsort(grp, kind="stable")
    gs = grp[perm]
    sparse_ok = bool((gs[:TBLK] == 0).all() and (gs[T - TBLK:] == 1).all())
    x = np.ascontiguousarray(x[perm])
    sel = sel[perm]
    rw = np.ascontiguousarray(rw[perm])

    xT_np = np.ascontiguousarray(
        x.T.reshape(NH // 2, 2, 128, T).transpose(0, 2, 1, 3)
    ).astype(NPBF16)                                  # [NH//2, 128, 2, T]

    # per-slot one-hot masks over the (e, r) axis, transposed to [ER, T];
    # applied HOST-side to the returned a2 (masking is elementwise, so it
    # commutes with the cross-core partial sum)
    masks = np.zeros((K, ER, T), dtype=np.float32)
    for k in range(K):
        onehot = np.zeros((T, E), np.float32)
        onehot[np.arange(T), sel[:, k]] = 1.0
        masks[k] = np.repeat(onehot, R, axis=1).T
    rwr_np = np.ascontiguousarray(rw.T).reshape(K, 1, T).astype(NPBF16)

    # flattened LoRA tensors (full copies; small)
    A1f = A1.reshape(ER, H)                      # [er, H]
    A3f = A3.reshape(ER, H)
    B2f = B2.transpose(0, 2, 1).reshape(ER, H)   # [er, H]

    # per-slot masked LoRA down-projections, computed host-side in fp32
    a1_all = x @ A1f.T.astype(np.float32)        # [T, ER]
    a3_all = x @ A3f.T.astype(np.float32)
    m1t_np = np.zeros((K, ER, T), dtype=NPBF16)
    m3t_np = np.zeros((K, ER, T), dtype=NPBF16)
    for k in range(K):
        mx = np.repeat(
            np.eye(E, dtype=np.float32)[sel[:, k]], R, axis=1)   # [T, ER]
        m1t_np[k] = (a1_all * mx).T.astype(NPBF16)
        m3t_np[k] = (a3_all * mx).T.astype(NPBF16)
    m1t_np = np.ascontiguousarray(
        m1t_np.reshape(K, NER, 128, T).transpose(0, 2, 1, 3))
    m3t_np = np.ascontiguousarray(
        m3t_np.reshape(K, NER, 128, T).transpose(0, 2, 1, 3))

    def pack_pairs(wT):
        # [H, FS] -> [NH//2, 128, 2*FS] with h=2p at cols [0:FS), h=2p+1
        # at [FS:2FS) (matches the kernel's paired w1t/w3t layout)
        return np.ascontiguousarray(
            wT.reshape(NH // 2, 2, 128, FS).transpose(0, 2, 1, 3)
        ).reshape(NH // 2, 128, 2 * FS)

    in_maps = []
    for c in range(NCORES):
        fs = slice(c * FS, (c + 1) * FS)
        w1T = np.ascontiguousarray(W1[fs].T).astype(NPBF16)   # [H, FS]
        w3T = np.ascontiguousarray(W3[fs].T).astype(NPBF16)
        w1t_np = pack_pairs(w1T)
        w3t_np = pack_pairs(w3T)
        w2T = np.ascontiguousarray(W2[:, fs].T).astype(NPBF16)  # [FS, H]
        w2t_np = w2T.reshape(NF, 128, H)
        b1f = B1[:, fs, :].transpose(0, 2, 1).reshape(ER, FS)   # [er, f]
        b3f = B3[:, fs, :].transpose(0, 2, 1).reshape(ER, FS)
        b1t_np = np.ascontiguousarray(b1f).astype(NPBF16).reshape(NER, 128, FS)
        b3t_np = np.ascontiguousarray(b3f).astype(NPBF16).reshape(NER, 128, FS)
        a2f = A2[:, :, fs].reshape(ER, FS)                      # [er, f]
        a2t_np = np.ascontiguousarray(a2f.T).astype(NPBF16).reshape(NF, 128, ER)

        in_maps.append({
            "xT": xT_np, "w1t": w1t_np, "w3t": w3t_np, "w2t": w2t_np,
            "m1t": m1t_np, "m3t": m3t_np, "b1t": b1t_np, "b3t": b3t_np,
            "a2t": a2t_np,
            "rwr": rwr_np,
        })
    return in_maps, (B2f.astype(np.float32), masks, perm, sparse_ok)


_CACHED_NC = {}


def kernel(hidden_states, Wg, W1, W2, W3, A1, B1, A2, B2, A3, B3,
           _trace=False, _tmpdir=None):
    in_maps, (B2f, masks, perm, sparse_ok) = prepare_inputs(
        hidden_states, Wg, W1, W2, W3, A1, B1, A2, B2, A3, B3)
    if sparse_ok not in _CACHED_NC:
        _CACHED_NC[sparse_ok] = build_nc(sparse=sparse_ok)
    nc = _CACHED_NC[sparse_ok]
    res = run_bass_kernel_spmd(nc, in_maps, list(range(NCORES)),
                               trace=_trace, tmpdir=_tmpdir)
    acc = np.zeros((NH, 128, T), np.float32)
    m2sum = np.zeros((K, ER, T), np.float32)
    for c in range(NCORES):
        acc += res.results[c]["outT"]
        m2sum += res.results[c]["m2o"].reshape(K, ER, T).astype(np.float32)
    out = acc.reshape(H, T)
    # host-side lora2: mask the (unmasked, core-summed) a2, then the final
    # LoRA up-projection is linear -> one small GEMM per slot
    for k in range(K):
        out += B2f.T @ (m2sum[k] * masks[k])
    outT_tok = out.T                       # [T, H], token-permuted order
    final = np.empty_like(outT_tok)
    final[perm] = outT_tok                 # undo the expert-group sort
    out = final.reshape(B, S, H)
    kernel.last_results = res
    return out


if __name__ == "__main__":
    nc = build_nc()
    print("built ok")

